# revision 9
# baseline (speedup 1.0000x reference)
"""Trainium2 Bass kernel for RelPatchAttention2D (THW).

Problem: q,k,v (4,16,16,128,128) f32. Patchify into 4096 patches/batch of
dim 1024. sim[q,k] = (qk+s)/(qq+kk-qk+s); tqk[k] = mean_q sim; out = tqk * v.

Sharding (no collectives): 8 cores = 4 batches x 2 key-halves. Each core:
full queries (4096) x its 2048 keys. Host prepares transposed patch
matrices (+augmentation rows), gathers/unpatchifies outputs.

Per-core kernel (layout: keys on partitions, queries on free dim):
  per (qt,kt) tile [128 keys x 512 queries]:
    PE:  8 fp32r matmuls (d-chunks) + 1 aug matmul (K=2: carries
         qq_q*1 + 1*(kk_k+s)) accumulate D = qq+kk+s-qk in PSUM
    DVE: r = reciprocal_approx_fast(D)
         N = (qqB + (kk+2s)_k) - D          (scalar_tensor_tensor)
         acc += sum_q N*r                   (tensor_tensor_reduce, chained)
  tqk = acc/4096; out = tqk * v  (ACT per-partition scale)

The N*r form keeps every error source relative to sim itself (avoids the
catastrophic cancellation of sum(A/D) - 4096).
"""
import os
import sys

import numpy as np

sys.path.insert(0, '/opt/trn_rl_repo')

SMOOTH = 1e-05
B, T, C, H, W = 4, 16, 16, 128, 128
SH = SW = 16
PH = PW = 8
NPATCH = T * SH * SW        # 4096 patches per batch (queries)
DPATCH = C * PH * PW        # 1024
KEYS_PER_CORE = NPATCH // 2  # 2048
N_CORES = 8

QT_TILES = NPATCH // 512     # 8
KT_TILES = KEYS_PER_CORE // 128  # 16
DC = DPATCH // 128           # 8 contraction chunks


# ----------------------------------------------------------------- host side

def _patchify_mat(x):
    # (B,T,C,H,W) -> (B, 4096, 1024), patch index = ((t*16+sh)*16+sw)
    xp = x.reshape(B, T, C, SH, PH, SW, PW).transpose(0, 1, 3, 5, 2, 4, 6)
    return np.ascontiguousarray(xp).reshape(B, NPATCH, DPATCH)


def _unpatchify_mat(p):
    # (B, 4096, 1024) -> (B,T,C,H,W)
    x = p.reshape(B, T, SH, SW, C, PH, PW).transpose(0, 1, 4, 2, 5, 3, 6)
    return np.ascontiguousarray(x).reshape(B, T, C, H, W)


def _rne11(x):
    """Round fp32 to 11 explicit mantissa bits, RNE — bit-exact replica of
    the PE's fp32r operand rounding (verified on HW both operands)."""
    i = np.asarray(x, np.float32).view(np.uint32).astype(np.uint64)
    bias = 0x7FF + ((i >> 12) & 1)
    return ((i + bias) & 0xFFFFF000).astype(np.uint32).view(np.float32)


def _host_prepare(q, k, v):
    QP = _patchify_mat(q)
    KP = _patchify_mat(k)
    VP = _patchify_mat(v)
    # qq/kk pre-rounded to the PE's fp32r grid so the aug matmul passes them
    # through EXACTLY; the same values feed the fp32 N-path (qqb/ck2), making
    # N = qk independent of fp32r quantization (which then only perturbs the
    # denominator D, a benign relative error).
    qq = _rne11(np.square(QP, dtype=np.float64).sum(-1).astype(np.float32))
    kk = _rne11(np.square(KP, dtype=np.float64).sum(-1).astype(np.float32))

    in_maps = []
    for b in range(B):
        QT = np.ascontiguousarray(QP[b].T)         # (1024, 4096)
        qta = np.concatenate(
            [QT,
             qq[b][None, :],
             np.ones((1, NPATCH), np.float32)], axis=0)   # (1026, 4096)
        qqb = np.broadcast_to(qq[b], (128, NPATCH)).copy()
        for half in range(2):
            sl = slice(half * KEYS_PER_CORE, (half + 1) * KEYS_PER_CORE)
            KT = np.ascontiguousarray(KP[b, sl].T)  # (1024, 2048)
            kta = np.concatenate(
                [-KT,
                 np.ones((1, KEYS_PER_CORE), np.float32),
                 kk[b, sl][None, :]], axis=0)
            ck2 = kk[b, sl][:, None]
            in_maps.append({
                'qta': qta,
                'kta': kta,
                'qqb': qqb,
                'ck2': np.ascontiguousarray(ck2),
                'vp': np.ascontiguousarray(VP[b, sl]),
            })
    return in_maps


def _host_finish(outs):
    full = np.empty((B, NPATCH, DPATCH), np.float32)
    for b in range(B):
        full[b, :KEYS_PER_CORE] = outs[2 * b]
        full[b, KEYS_PER_CORE:] = outs[2 * b + 1]
    return _unpatchify_mat(full)


# --------------------------------------------------------------- bass kernel

def build_nc():
    import concourse.bass as bass  # noqa: F401
    import concourse.mybir as mybir
    import concourse.tile as tile
    from concourse import bacc

    f32 = mybir.dt.float32
    f32r = mybir.dt.float32r
    Alu = mybir.AluOpType
    Act = mybir.ActivationFunctionType

    nc = bacc.Bacc(
        "TRN2",
        target_bir_lowering=False,
        debug=False,
        enable_asserts=False,
        num_devices=N_CORES,
    )

    qta = nc.dram_tensor("qta", [DPATCH + 2, NPATCH], f32r, kind="ExternalInput").ap()
    kta = nc.dram_tensor("kta", [DPATCH + 2, KEYS_PER_CORE], f32r, kind="ExternalInput").ap()
    qqb = nc.dram_tensor("qqb", [128, NPATCH], f32, kind="ExternalInput").ap()
    ck2 = nc.dram_tensor("ck2", [KEYS_PER_CORE, 1], f32, kind="ExternalInput").ap()
    vp = nc.dram_tensor("vp", [KEYS_PER_CORE, DPATCH], f32, kind="ExternalInput").ap()
    out = nc.dram_tensor("out", [KEYS_PER_CORE, DPATCH], f32, kind="ExternalOutput").ap()

    with tile.TileContext(nc) as tc:
        with (
            tc.tile_pool(name="ktp", bufs=1) as ktp,
            tc.tile_pool(name="ckp", bufs=1) as ckp,
            tc.tile_pool(name="qp", bufs=2) as qp,
            tc.tile_pool(name="qqbp", bufs=2) as qqbp,
            tc.tile_pool(name="psp", bufs=4, space="PSUM") as psp,
            tc.tile_pool(name="rp", bufs=3) as rp,
            tc.tile_pool(name="np_", bufs=3) as np_p,
            tc.tile_pool(name="scrp", bufs=2) as scrp,
            tc.tile_pool(name="accp", bufs=1) as accp,
            tc.tile_pool(name="wp", bufs=2) as wp,
            tc.tile_pool(name="vvp", bufs=3) as vvp,
            tc.tile_pool(name="outp", bufs=3) as outp,
        ):
            # resident K^T chunks (negated) + aug rows
            kt_tiles = []
            for c in range(DC):
                t = ktp.tile([128, KEYS_PER_CORE], f32r, name=f"ktt{c}", tag=f"ktt{c}")
                nc.sync.dma_start(t[:], kta[c * 128:(c + 1) * 128, :])
                kt_tiles.append(t)
            kt_aug = ktp.tile([2, KEYS_PER_CORE], f32r, name="ktaug", tag="ktaug")
            nc.sync.dma_start(kt_aug[:], kta[DPATCH:DPATCH + 2, :])

            ck_tiles = []
            for kt in range(KT_TILES):
                t = ckp.tile([128, 1], f32, name=f"ck{kt}", tag=f"ck{kt}")
                nc.sync.dma_start(t[:], ck2[kt * 128:(kt + 1) * 128, :])
                ck_tiles.append(t)

            # per-kt accumulators: one column per qt, reduced at the end
            acc_tiles = []
            for kt in range(KT_TILES):
                t = accp.tile([128, QT_TILES], f32, name=f"acc{kt}", tag=f"acc{kt}")
                acc_tiles.append(t)

            for qt in range(QT_TILES):
                qs = slice(qt * 512, (qt + 1) * 512)
                q_tiles = []
                for c in range(DC):
                    t = qp.tile([128, 512], f32r, name=f"qtt{c}_{qt}", tag=f"qtt{c}")
                    nc.sync.dma_start(t[:], qta[c * 128:(c + 1) * 128, qs])
                    q_tiles.append(t)
                q_aug = qp.tile([2, 512], f32r, name=f"qaug_{qt}", tag="qaug")
                nc.sync.dma_start(q_aug[:], qta[DPATCH:DPATCH + 2, qs])
                qqb_t = qqbp.tile([128, 512], f32, name=f"qqb_{qt}", tag="qqb")
                nc.sync.dma_start(qqb_t[:], qqb[:, qs])

                for kt in range(KT_TILES):
                    ks = slice(kt * 128, (kt + 1) * 128)
                    ps = psp.tile([128, 512], f32, name=f"ps_{qt}_{kt}", tag="ps")
                    for c in range(DC):
                        nc.tensor.matmul(
                            ps[:],
                            kt_tiles[c][:, ks],
                            q_tiles[c][:],
                            start=(c == 0),
                            stop=False,
                        )
                    nc.tensor.matmul(
                        ps[:],
                        kt_aug[:, ks],
                        q_aug[:],
                        start=False,
                        stop=True,
                    )

                    r_t = rp.tile([128, 512], f32, name=f"r_{qt}_{kt}", tag="r")
                    nc.vector.reciprocal_approx_fast(r_t[:], ps[:])

                    n_t = np_p.tile([128, 512], f32, name=f"n_{qt}_{kt}", tag="n")
                    nc.vector.scalar_tensor_tensor(
                        n_t[:], qqb_t[:], ck_tiles[kt][:], ps[:],
                        op0=Alu.add, op1=Alu.subtract,
                    )

                    scr = scrp.tile([128, 512], f32, name=f"scr_{qt}_{kt}", tag="scr")
                    nc.vector.scalar_tensor_tensor(
                        scr[:], n_t[:], 1.0, r_t[:],
                        op0=Alu.bypass, op1=Alu.mult,
                        accum_out=acc_tiles[kt][:, qt:qt + 1],
                    )

                    if qt == QT_TILES - 1:
                        red_t = wp.tile([128, 1], f32, name=f"red_{kt}", tag="red")
                        nc.vector.tensor_reduce(
                            red_t[:], acc_tiles[kt][:],
                            op=Alu.add, axis=mybir.AxisListType.X)
                        w_t = wp.tile([128, 1], f32, name=f"w_{kt}", tag="w")
                        nc.scalar.activation(
                            w_t[:], red_t[:], Act.Copy, scale=1.0 / NPATCH)
                        v_t = vvp.tile([128, DPATCH], f32, name=f"v_{kt}", tag="v")
                        nc.sync.dma_start(v_t[:], vp[ks, :])
                        o_t = outp.tile([128, DPATCH], f32, name=f"o_{kt}", tag="o")
                        nc.scalar.activation(o_t[:], v_t[:], Act.Copy, scale=w_t[:])
                        nc.sync.dma_start(out[ks, :], o_t[:])

    nc.compile()
    return nc


_NC_CACHE = None


def _get_nc():
    global _NC_CACHE
    if _NC_CACHE is None:
        _NC_CACHE = build_nc()
    return _NC_CACHE


# ---------------------------------------------------------------- entrypoint

def kernel(q, k, v, _trace=False):
    q = np.asarray(q, dtype=np.float32)
    k = np.asarray(k, dtype=np.float32)
    v = np.asarray(v, dtype=np.float32)

    in_maps = _host_prepare(q, k, v)
    nc = _get_nc()

    from concourse.bass_utils import run_bass_kernel_spmd
    res = run_bass_kernel_spmd(
        nc, in_maps, core_ids=list(range(N_CORES)), trace=_trace)
    outs = [r['out'] for r in res.results]
    result = _host_finish(outs)
    if _trace:
        kernel.last_results = res
    return result


if __name__ == '__main__':
    rng = np.random.default_rng(0)
    q = rng.standard_normal((B, T, C, H, W), dtype=np.float32)
    k = rng.standard_normal((B, T, C, H, W), dtype=np.float32)
    v = rng.standard_normal((B, T, C, H, W), dtype=np.float32)
    o = kernel(q, k, v)
    print("out", o.shape, o.dtype, float(np.abs(o).mean()))


# revision 12
# speedup vs baseline: 1.1863x; 1.1863x over previous
"""Trainium2 Bass kernel for RelPatchAttention2D (THW).

Problem: q,k,v (4,16,16,128,128) f32. Patchify into 4096 patches/batch of
dim 1024. sim[q,k] = (qk+s)/(qq+kk-qk+s); tqk[k] = mean_q sim; out = tqk * v.

Sharding (no collectives): 8 cores = 4 batches x 2 key-halves. Each core:
full queries (4096) x its 2048 keys. Host prepares transposed patch
matrices (+augmentation rows), gathers/unpatchifies outputs.

Per-core kernel (layout: keys on partitions, queries on free dim):
  per (qt,kt) tile [128 keys x 512 queries]:
    PE:  8 matmuls (d-chunks; stationary -K^T fp32r, moving Q^T bf16)
         accumulate P = -qk in PSUM
    ACT: N = -P + s   (PSUM->SBUF, scale=-1, bias=s)  [numerator qk+s]
    PE:  1 aug matmul (K=2 rows: qq_q*1 + 1*kk_k) onto the same bank
         -> D = qq+kk-qk   (denominator; s vanishes below fp32r ulp)
    DVE: r = reciprocal_approx_fast(D)
         acc[:,qt] = sum_q N*r   (scalar_tensor_tensor accum)
  tqk = rowsum(acc)/4096; out = tqk * v  (ACT per-partition scale)

Numerics: N comes from the PSUM qk directly, so qq/kk quantization (bf16
grid for qq on the moving side, fp32r 11-bit grid for kk) only perturbs
the denominator - a benign RELATIVE error on sim. The N*r form keeps the
reciprocal's error relative to sim as well (no catastrophic cancellation).
"""
import os
import sys

import numpy as np

sys.path.insert(0, '/opt/trn_rl_repo')

SMOOTH = 1e-05
B, T, C, H, W = 4, 16, 16, 128, 128
SH = SW = 16
PH = PW = 8
NPATCH = T * SH * SW        # 4096 patches per batch (queries)
DPATCH = C * PH * PW        # 1024
KEYS_PER_CORE = NPATCH // 2  # 2048
N_CORES = 8

QT_TILES = NPATCH // 512     # 8
KT_TILES = KEYS_PER_CORE // 128  # 16
DC = DPATCH // 128           # 8 contraction chunks


# ----------------------------------------------------------------- host side

def _patchify_mat(x):
    # (B,T,C,H,W) -> (B, 4096, 1024), patch index = ((t*16+sh)*16+sw)
    xp = x.reshape(B, T, C, SH, PH, SW, PW).transpose(0, 1, 3, 5, 2, 4, 6)
    return np.ascontiguousarray(xp).reshape(B, NPATCH, DPATCH)


def _unpatchify_mat(p):
    # (B, 4096, 1024) -> (B,T,C,H,W)
    x = p.reshape(B, T, SH, SW, C, PH, PW).transpose(0, 1, 4, 2, 5, 3, 6)
    return np.ascontiguousarray(x).reshape(B, T, C, H, W)


def _rne11(x):
    """Round fp32 to 11 explicit mantissa bits, RNE — bit-exact replica of
    the PE's fp32r operand rounding (verified on HW, both operands)."""
    i = np.asarray(x, np.float32).view(np.uint32).astype(np.uint64)
    bias = 0x7FF + ((i >> 12) & 1)
    return ((i + bias) & 0xFFFFF000).astype(np.uint32).view(np.float32)


def _host_prepare(q, k, v):
    import ml_dtypes
    QP = _patchify_mat(q)
    KP = _patchify_mat(k)
    VP = _patchify_mat(v)
    qq = np.square(QP, dtype=np.float64).sum(-1).astype(np.float32)
    kk = np.square(KP, dtype=np.float64).sum(-1).astype(np.float32)

    in_maps = []
    for b in range(B):
        QT = np.ascontiguousarray(QP[b].T)         # (1024, 4096) f32
        qta = np.concatenate(
            [QT,
             qq[b][None, :],
             np.ones((1, NPATCH), np.float32)], axis=0)   # (1026, 4096)
        qta = qta.astype(ml_dtypes.bfloat16)       # bf16 matmul operands
        for half in range(2):
            sl = slice(half * KEYS_PER_CORE, (half + 1) * KEYS_PER_CORE)
            KT = np.ascontiguousarray(KP[b, sl].T)  # (1024, 2048)
            kta = np.concatenate(
                [-KT,
                 np.ones((1, KEYS_PER_CORE), np.float32),
                 kk[b, sl][None, :]], axis=0).astype(ml_dtypes.bfloat16)
            in_maps.append({
                'qta': qta,
                'kta': kta,
                'vp': np.ascontiguousarray(VP[b, sl]),
            })
    return in_maps


def _host_finish(outs):
    full = np.empty((B, NPATCH, DPATCH), np.float32)
    for b in range(B):
        full[b, :KEYS_PER_CORE] = outs[2 * b]
        full[b, KEYS_PER_CORE:] = outs[2 * b + 1]
    return _unpatchify_mat(full)


# --------------------------------------------------------------- bass kernel

def build_nc():
    import concourse.bass as bass  # noqa: F401
    import concourse.mybir as mybir
    import concourse.tile as tile
    from concourse import bacc

    f32 = mybir.dt.float32
    f32r = mybir.dt.float32r
    bf16 = mybir.dt.bfloat16
    Alu = mybir.AluOpType
    Act = mybir.ActivationFunctionType

    nc = bacc.Bacc(
        "TRN2",
        target_bir_lowering=False,
        debug=False,
        enable_asserts=False,
        num_devices=N_CORES,
    )

    qta = nc.dram_tensor("qta", [DPATCH + 2, NPATCH], bf16, kind="ExternalInput").ap()
    kta = nc.dram_tensor("kta", [DPATCH + 2, KEYS_PER_CORE], bf16, kind="ExternalInput").ap()
    vp = nc.dram_tensor("vp", [KEYS_PER_CORE, DPATCH], f32, kind="ExternalInput").ap()
    out = nc.dram_tensor("out", [KEYS_PER_CORE, DPATCH], f32, kind="ExternalOutput").ap()

    with tile.TileContext(nc) as tc:
        with (
            tc.tile_pool(name="ktp", bufs=1) as ktp,
            tc.tile_pool(name="qp", bufs=2) as qp,
            tc.tile_pool(name="psp", bufs=4, space="PSUM") as psp,
            tc.tile_pool(name="rp", bufs=3) as rp,
            tc.tile_pool(name="np_", bufs=3) as np_p,
            tc.tile_pool(name="scrp", bufs=2) as scrp,
            tc.tile_pool(name="accp", bufs=1) as accp,
            tc.tile_pool(name="wp", bufs=2) as wp,
            tc.tile_pool(name="vvp", bufs=3) as vvp,
            tc.tile_pool(name="outp", bufs=3) as outp,
        ):
            # resident K^T chunks (negated) + aug rows [ones; kk]
            kt_tiles = []
            for c in range(DC):
                t = ktp.tile([128, KEYS_PER_CORE], bf16, name=f"ktt{c}", tag=f"ktt{c}")
                nc.sync.dma_start(t[:], kta[c * 128:(c + 1) * 128, :])
                kt_tiles.append(t)
            kt_aug = ktp.tile([2, KEYS_PER_CORE], bf16, name="ktaug", tag="ktaug")
            nc.sync.dma_start(kt_aug[:], kta[DPATCH:DPATCH + 2, :])

            # per-kt accumulators: one column per qt, reduced at the end
            acc_tiles = []
            for kt in range(KT_TILES):
                t = accp.tile([128, QT_TILES], f32, name=f"acc{kt}", tag=f"acc{kt}")
                acc_tiles.append(t)

            for qt in range(QT_TILES):
                qs = slice(qt * 512, (qt + 1) * 512)
                q_tiles = []
                for c in range(DC):
                    t = qp.tile([128, 512], bf16, name=f"qtt{c}_{qt}", tag=f"qtt{c}")
                    nc.sync.dma_start(t[:], qta[c * 128:(c + 1) * 128, qs])
                    q_tiles.append(t)
                q_aug = qp.tile([2, 512], bf16, name=f"qaug_{qt}", tag="qaug")
                nc.sync.dma_start(q_aug[:], qta[DPATCH:DPATCH + 2, qs])

                for kt in range(KT_TILES):
                    ks = slice(kt * 128, (kt + 1) * 128)
                    ps = psp.tile([128, 512], f32, name=f"ps_{qt}_{kt}", tag="ps")
                    # P = -qk
                    for c in range(DC):
                        nc.tensor.matmul(
                            ps[:],
                            kt_tiles[c][:, ks],
                            q_tiles[c][:],
                            start=(c == 0),
                            stop=(c == DC - 1),
                        )
                    # numerator N = qk + s, read before the aug matmul
                    n_t = np_p.tile([128, 512], f32, name=f"n_{qt}_{kt}", tag="n")
                    nc.scalar.activation(
                        n_t[:], ps[:], Act.Copy, bias=SMOOTH, scale=-1.0)
                    # denominator: D = qq + kk - qk accumulated onto P
                    nc.tensor.matmul(
                        ps[:],
                        kt_aug[:, ks],
                        q_aug[:],
                        start=False,
                        stop=True,
                        skip_group_check=True,
                    )

                    r_t = rp.tile([128, 512], f32, name=f"r_{qt}_{kt}", tag="r")
                    nc.vector.reciprocal_approx_fast(r_t[:], ps[:])

                    scr = scrp.tile([128, 512], f32, name=f"scr_{qt}_{kt}", tag="scr")
                    nc.vector.scalar_tensor_tensor(
                        scr[:], n_t[:], 1.0, r_t[:],
                        op0=Alu.bypass, op1=Alu.mult,
                        accum_out=acc_tiles[kt][:, qt:qt + 1],
                    )

                    if qt == QT_TILES - 1:
                        red_t = wp.tile([128, 1], f32, name=f"red_{kt}", tag="red")
                        nc.vector.tensor_reduce(
                            red_t[:], acc_tiles[kt][:],
                            op=Alu.add, axis=mybir.AxisListType.X)
                        w_t = wp.tile([128, 1], f32, name=f"w_{kt}", tag="w")
                        nc.scalar.activation(
                            w_t[:], red_t[:], Act.Copy, scale=1.0 / NPATCH)
                        v_t = vvp.tile([128, DPATCH], f32, name=f"v_{kt}", tag="v")
                        nc.sync.dma_start(v_t[:], vp[ks, :])
                        o_t = outp.tile([128, DPATCH], f32, name=f"o_{kt}", tag="o")
                        nc.scalar.activation(o_t[:], v_t[:], Act.Copy, scale=w_t[:])
                        nc.sync.dma_start(out[ks, :], o_t[:])

    nc.compile()
    return nc


_NC_CACHE = None


def _get_nc():
    global _NC_CACHE
    if _NC_CACHE is None:
        _NC_CACHE = build_nc()
    return _NC_CACHE


# ---------------------------------------------------------------- entrypoint

def kernel(q, k, v, _trace=False):
    q = np.asarray(q, dtype=np.float32)
    k = np.asarray(k, dtype=np.float32)
    v = np.asarray(v, dtype=np.float32)

    in_maps = _host_prepare(q, k, v)
    nc = _get_nc()

    from concourse.bass_utils import run_bass_kernel_spmd
    res = run_bass_kernel_spmd(
        nc, in_maps, core_ids=list(range(N_CORES)), trace=_trace)
    outs = [r['out'] for r in res.results]
    result = _host_finish(outs)
    if _trace:
        kernel.last_results = res
    return result


if __name__ == '__main__':
    rng = np.random.default_rng(0)
    q = rng.standard_normal((B, T, C, H, W), dtype=np.float32)
    k = rng.standard_normal((B, T, C, H, W), dtype=np.float32)
    v = rng.standard_normal((B, T, C, H, W), dtype=np.float32)
    o = kernel(q, k, v)
    print("out", o.shape, o.dtype, float(np.abs(o).mean()))
